# revision 17
# baseline (speedup 1.0000x reference)
"""Trainium2 Bass kernel for nn_NeuralNetwork_42528766165249 (DEQ GRU + Broyden).

Math: reference Broyden solver converges at the plain Picard contraction rate
(measured rate ~0.56/iter, 11 iters, monotone); K=16 Picard iterations of
z <- tanh(GRU_z(z) + z0) reproduce the reference output to ~2.5e-4 rel err.

Sharding: data-parallel over batch (B=64 -> 8 cores x 8). Per core:
  phase 1: sequential GRU_x scan over S=128 producing z0 (stored transposed).
  phase 2: K=16 Picard iterations wavefront-pipelined: lane (k,b) at diagonal
           step d processes timestep t=d-k; all 16x8=128 lanes share one
           M=128 fused matmul  [z_prev; h] @ [Wih_z; Whh_z]^T  (f32r, full PE).
  phase 3: head out[b] = sum(z * Wfc) + bfc via DVE reduce + PE partition-sum.
"""
import numpy as np
import concourse.bass as bass
import concourse.bacc as bacc
import concourse.mybir as mybir
import concourse.tile as tile
from concourse.bass import AP
from concourse.bass_utils import run_bass_kernel_spmd
from concourse.masks import make_identity

F32 = mybir.dt.float32
F32R = mybir.dt.float32r
NCORE = 8
B, S, D, H = 64, 128, 128, 512
BS = B // NCORE          # 8 batch per core
K = 16                   # picard iterations (= wavefront lanes / BS)
NL = K * BS              # 128 lanes
TT = S + K - 1           # 143 wavefront steps
ZT = S + 2 * (K - 1)     # z0T time slots (tt = t + K-1, t in [-(K-1), 127+K-1])
TOFF = K - 1             # 15


def r32(ap):
    return ap.bitcast(F32R)


def build_nc():
    nc = bacc.Bacc("TRN2", target_bir_lowering=False, debug=False,
                   num_devices=NCORE)
    dt = F32
    # inputs (per-core xT differs; weights replicated)
    xT = nc.dram_tensor("xT", [128, S, BS], dt, kind="ExternalInput")
    w_rz_x = nc.dram_tensor("w_rz_x", [128, 5, 1024], dt, kind="ExternalInput")
    w_ni_x = nc.dram_tensor("w_ni_x", [128, 1, 512], dt, kind="ExternalInput")
    w_nh_x = nc.dram_tensor("w_nh_x", [128, 4, 512], dt, kind="ExternalInput")
    b_rz_x = nc.dram_tensor("b_rz_x", [1, 1024], dt, kind="ExternalInput")
    b_ni_x = nc.dram_tensor("b_ni_x", [1, 512], dt, kind="ExternalInput")
    b_nh_x = nc.dram_tensor("b_nh_x", [1, 512], dt, kind="ExternalInput")
    w_rz = nc.dram_tensor("w_rz", [128, 8, 1024], dt, kind="ExternalInput")
    w_ni = nc.dram_tensor("w_ni", [128, 4, 512], dt, kind="ExternalInput")
    w_nh = nc.dram_tensor("w_nh", [128, 4, 512], dt, kind="ExternalInput")
    b_rz = nc.dram_tensor("b_rz", [1, 1024], dt, kind="ExternalInput")
    b_ni = nc.dram_tensor("b_ni", [1, 512], dt, kind="ExternalInput")
    b_nh = nc.dram_tensor("b_nh", [1, 512], dt, kind="ExternalInput")
    wfcT = nc.dram_tensor("wfcT", [128, 4, S], dt, kind="ExternalInput")
    bfc_r = nc.dram_tensor("bfc_r", [BS, 1], dt, kind="ExternalInput")
    hmask = nc.dram_tensor("hmask", [128, K + 1], dt, kind="ExternalInput")
    out_e = nc.dram_tensor("out", [BS, 1], dt, kind="ExternalOutput")

    Sig = mybir.ActivationFunctionType.Sigmoid
    Tanh = mybir.ActivationFunctionType.Tanh

    with tile.TileContext(nc) as tc:
        with tc.tile_pool(name="const", bufs=1) as cpool:
            # persistent SBUF
            ident = cpool.tile([128, 128], dt, tag="ident")
            make_identity(nc, ident[:])
            ones = cpool.tile([1, 128], dt, tag="ones")
            nc.vector.memset(ones[:], 1.0)
            ones_col = cpool.tile([128, 1], dt, tag="ones_col")
            nc.vector.memset(ones_col[:], 1.0)
            sw_rz_x = cpool.tile([128, 5, 1024], dt, tag="w_rz_x")
            nc.gpsimd.dma_start(r32(sw_rz_x[:]), w_rz_x[:])
            sw_ni_x = cpool.tile([128, 1, 512], dt, tag="w_ni_x")
            nc.gpsimd.dma_start(r32(sw_ni_x[:]), w_ni_x[:])
            sw_nh_x = cpool.tile([128, 4, 512], dt, tag="w_nh_x")
            nc.gpsimd.dma_start(r32(sw_nh_x[:]), w_nh_x[:])
            sb_rz_x = cpool.tile([1, 1024], dt, tag="b_rz_x")
            nc.gpsimd.dma_start(r32(sb_rz_x[:]), b_rz_x[:])
            sb_ni_x = cpool.tile([1, 512], dt, tag="b_ni_x")
            nc.gpsimd.dma_start(r32(sb_ni_x[:]), b_ni_x[:])
            sb_nh_x = cpool.tile([1, 512], dt, tag="b_nh_x")
            nc.gpsimd.dma_start(r32(sb_nh_x[:]), b_nh_x[:])
            sw_rz = cpool.tile([128, 8, 1024], dt, tag="w_rz")
            nc.gpsimd.dma_start(r32(sw_rz[:]), w_rz[:])
            sw_ni = cpool.tile([128, 4, 512], dt, tag="w_ni")
            nc.gpsimd.dma_start(r32(sw_ni[:]), w_ni[:])
            sw_nh = cpool.tile([128, 4, 512], dt, tag="w_nh")
            nc.gpsimd.dma_start(r32(sw_nh[:]), w_nh[:])
            sb_rz = cpool.tile([1, 1024], dt, tag="b_rz")
            nc.gpsimd.dma_start(r32(sb_rz[:]), b_rz[:])
            sb_ni = cpool.tile([1, 512], dt, tag="b_ni")
            nc.gpsimd.dma_start(r32(sb_ni[:]), b_ni[:])
            sb_nh = cpool.tile([1, 512], dt, tag="b_nh")
            nc.gpsimd.dma_start(r32(sb_nh[:]), b_nh[:])
            sxT = cpool.tile([128, S, BS], dt, tag="xT")
            nc.gpsimd.dma_start(r32(sxT[:]), xT[:])
            swfcT = cpool.tile([128, 4, S], dt, tag="wfcT")
            nc.sync.dma_start(swfcT[:], wfcT[:])
            sbfc = cpool.tile([BS, 1], dt, tag="bfc")
            nc.sync.dma_start(sbfc[:], bfc_r[:])
            shmask = cpool.tile([128, K + 1], dt, tag="hmask")
            nc.sync.dma_start(shmask[:], hmask[:])
            # z0 transposed store: [p, c, tt, b], tt = t + TOFF
            z0T = cpool.tile([128, 4, ZT, BS], dt, tag="z0T")
            nc.vector.memset(z0T[:, :, 0:TOFF, :], 0.0)  # junk/initial region
            nc.vector.memset(z0T[:, :, S + TOFF:ZT, :], 0.0)  # junk tail
            # final picard iterate, T layout [p, c, t, b]
            zfin = cpool.tile([128, 4, S, BS], dt, tag="zfin")

            # ---------------- phase 1: GRU_x scan (BS lanes) ----------------
            with (
                tc.tile_pool(name="p1s", bufs=2) as p1s,
                tc.tile_pool(name="p1rz", bufs=1, space="PSUM") as p1rz,
                tc.tile_pool(name="p1n", bufs=1, space="PSUM") as p1n,
                tc.tile_pool(name="p1t", bufs=2, space="PSUM") as p1t,
            ):
                h_lane = p1s.tile([BS, 512], dt, tag="h1")
                nc.vector.memset(h_lane[:], 0.0)
                for t in range(S):
                    rz_ps = p1rz.tile([BS, 1024], dt, tag="rz1")
                    n_ps = p1n.tile([BS, 1024], dt, tag="n1")  # [ni | nh]
                    xs = r32(sxT[:, t, :])
                    hs = [r32(z0T[:, c, t - 1 + TOFF, :]) for c in range(4)]
                    for n in range(2):
                        nsl = slice(512 * n, 512 * n + 512)
                        nc.tensor.matmul(rz_ps[:, nsl], xs,
                                         r32(sw_rz_x[:, 0, nsl]),
                                         start=True, stop=False)
                        for j in range(4):
                            nc.tensor.matmul(rz_ps[:, nsl], hs[j],
                                             r32(sw_rz_x[:, 1 + j, nsl]),
                                             start=False, stop=False)
                        nc.tensor.matmul(rz_ps[:, nsl], r32(ones[0:1, 0:BS]),
                                         r32(sb_rz_x[0:1, nsl]),
                                         start=False, stop=True)
                    nc.tensor.matmul(n_ps[:, 0:512], xs, r32(sw_ni_x[:, 0, :]),
                                     start=True, stop=False)
                    nc.tensor.matmul(n_ps[:, 0:512], r32(ones[0:1, 0:BS]),
                                     r32(sb_ni_x[0:1, :]), start=False, stop=True)
                    for j in range(4):
                        nc.tensor.matmul(n_ps[:, 512:1024], hs[j],
                                         r32(sw_nh_x[:, j, :]),
                                         start=(j == 0), stop=False)
                    nc.tensor.matmul(n_ps[:, 512:1024], r32(ones[0:1, 0:BS]),
                                     r32(sb_nh_x[0:1, :]), start=False, stop=True)
                    # gates
                    r_sb = p1s.tile([BS, 512], dt, tag="r1")
                    zg_sb = p1s.tile([BS, 512], dt, tag="zg1")
                    nc.scalar.activation(r_sb[:], rz_ps[:, 0:512], Sig)
                    nc.scalar.activation(zg_sb[:], rz_ps[:, 512:1024], Sig)
                    t1 = p1s.tile([BS, 512], dt, tag="t1a")
                    nc.vector.tensor_mul(t1[:], r_sb[:], n_ps[:, 512:1024])
                    nsum = p1s.tile([BS, 512], dt, tag="t1b")
                    nc.vector.tensor_add(nsum[:], t1[:], n_ps[:, 0:512])
                    n_sb = p1s.tile([BS, 512], dt, tag="n1s")
                    nc.scalar.activation(n_sb[:], nsum[:], Tanh)
                    hmn = p1s.tile([BS, 512], dt, tag="hmn1")
                    nc.gpsimd.tensor_sub(hmn[:], h_lane[:], n_sb[:])
                    u = p1s.tile([BS, 512], dt, tag="u1")
                    nc.gpsimd.tensor_mul(u[:], hmn[:], zg_sb[:])
                    h_new = p1s.tile([BS, 512], dt, tag="h1")
                    nc.vector.tensor_add(h_new[:], u[:], n_sb[:])
                    # transpose h_new -> z0T[:, :, t+TOFF, :]
                    ht_ps = p1t.tile([128, 4, BS], dt, tag="ht1")
                    for c in range(4):
                        nc.tensor.transpose(ht_ps[:, c, :],
                                            h_new[:, 128 * c:128 * c + 128],
                                            ident[0:BS, 0:BS])
                    nc.vector.tensor_copy(r32(z0T[:, :, t + TOFF, :]), ht_ps[:])
                    h_lane = h_new

            # ---------------- phase 2: picard wavefront ----------------
            with (
                tc.tile_pool(name="p2s", bufs=2) as p2s,
                tc.tile_pool(name="p2w", bufs=3) as p2w,
                tc.tile_pool(name="p2rz", bufs=2, space="PSUM") as p2rz,
                tc.tile_pool(name="p2ni", bufs=1, space="PSUM") as p2ni,
                tc.tile_pool(name="p2nh", bufs=1, space="PSUM") as p2nh,
                tc.tile_pool(name="p2t", bufs=2, space="PSUM") as p2t,
            ):
                zT_cur = p2s.tile([128, 4, K, BS], dt, tag="zT")
                nc.vector.memset(zT_cur[:], 0.0)
                nc.vector.tensor_copy(r32(zT_cur[:, :, 0, :]), z0T[:, :, TOFF, :])
                hT_cur = p2s.tile([128, 4, K, BS], dt, tag="hT")
                nc.vector.memset(hT_cur[:], 0.0)
                h_lane = p2s.tile([128, 512], dt, tag="h2")
                nc.vector.memset(h_lane[:], 0.0)
                for d in range(TT):
                    rz_ps = p2rz.tile([128, 1024], dt, tag="rz2")
                    ni_ps = p2ni.tile([128, 512], dt, tag="ni2")
                    nh_ps = p2nh.tile([128, 512], dt, tag="nh2")
                    stat = ([r32(zT_cur[:, c, :, :]) for c in range(4)]
                            + [r32(hT_cur[:, c, :, :]) for c in range(4)])
                    for n in range(2):
                        nsl = slice(512 * n, 512 * n + 512)
                        for j in range(8):
                            nc.tensor.matmul(rz_ps[:, nsl], stat[j],
                                             r32(sw_rz[:, j, nsl]),
                                             start=(j == 0), stop=False)
                        nc.tensor.matmul(rz_ps[:, nsl], r32(ones[0:1, :]),
                                         r32(sb_rz[0:1, nsl]),
                                         start=False, stop=True)
                    for j in range(4):
                        nc.tensor.matmul(ni_ps[:], stat[j], r32(sw_ni[:, j, :]),
                                         start=(j == 0), stop=False)
                    nc.tensor.matmul(ni_ps[:], r32(ones[0:1, :]),
                                     r32(sb_ni[0:1, :]), start=False, stop=True)
                    for j in range(4):
                        nc.tensor.matmul(nh_ps[:], stat[4 + j],
                                         r32(sw_nh[:, j, :]),
                                         start=(j == 0), stop=False)
                    nc.tensor.matmul(nh_ps[:], r32(ones[0:1, :]),
                                     r32(sb_nh[0:1, :]), start=False, stop=True)
                    # gates / state update (lane layout)
                    r_sb = p2w.tile([128, 512], dt, tag="r2")
                    zg_sb = p2w.tile([128, 512], dt, tag="zg2")
                    nc.scalar.activation(r_sb[:], rz_ps[:, 0:512], Sig)
                    nc.scalar.activation(zg_sb[:], rz_ps[:, 512:1024], Sig)
                    t1 = p2w.tile([128, 512], dt, tag="t2a")
                    nc.vector.tensor_mul(t1[:], r_sb[:], nh_ps[:])
                    nsum = p2w.tile([128, 512], dt, tag="t2b")
                    nc.vector.tensor_add(nsum[:], t1[:], ni_ps[:])
                    n_sb = p2w.tile([128, 512], dt, tag="n2s")
                    nc.scalar.activation(n_sb[:], nsum[:], Tanh)
                    hmn = p2w.tile([128, 512], dt, tag="hmn2")
                    jm = min(d, K)
                    nc.vector.scalar_tensor_tensor(
                        hmn[:], h_lane[:], shmask[:, jm:jm + 1], n_sb[:],
                        op0=mybir.AluOpType.mult,
                        op1=mybir.AluOpType.subtract)
                    u = p2w.tile([128, 512], dt, tag="u2")
                    nc.gpsimd.tensor_mul(u[:], hmn[:], zg_sb[:])
                    h_new = p2s.tile([128, 512], dt, tag="h2")
                    nc.vector.tensor_add(h_new[:], u[:], n_sb[:])
                    # transpose h_new -> T layout psum
                    ht_ps = p2t.tile([128, 4, 128], dt, tag="ht2")
                    for c in range(4):
                        nc.tensor.transpose(ht_ps[:, c, :],
                                            h_new[:, 128 * c:128 * c + 128],
                                            ident[:])
                    # z_pre = h_T + z0T diag ;  z_out = tanh(z_pre)
                    zpre = p2w.tile([128, 4, K, BS], dt, tag="zpre")
                    sl = slice(d + TOFF, d - 1, -1) if d >= 1 else \
                        slice(TOFF, None, -1)
                    nc.vector.tensor_add(
                        zpre[:], ht_ps[:].rearrange("p c (k b) -> p c k b", b=BS),
                        z0T[:, :, sl, :])
                    zT_nxt = p2s.tile([128, 4, K, BS], dt, tag="zT")
                    nc.scalar.activation(r32(zT_nxt[:, :, 1:K, :]),
                                         zpre[:, :, 0:K - 1, :], Tanh)
                    if d >= TOFF:
                        nc.scalar.activation(zfin[:, :, d - TOFF, :],
                                             zpre[:, :, K - 1, :], Tanh)
                    if d + 1 < S:
                        nc.vector.tensor_copy(r32(zT_nxt[:, :, 0, :]),
                                              z0T[:, :, d + 1 + TOFF, :])
                    else:
                        nc.vector.memset(zT_nxt[:, :, 0, :], 0.0)
                    hT_nxt = p2s.tile([128, 4, K, BS], dt, tag="hT")
                    nc.vector.tensor_copy(
                        r32(hT_nxt[:]), ht_ps[:].rearrange("p c (k b) -> p c k b", b=BS))
                    if d + 1 < K:
                        # lane k=d+1 starts at step d+1 with h=0 (T side;
                        # lane-layout side handled by hmask in hmn)
                        nc.vector.memset(hT_nxt[:, :, d + 1, :], 0.0)
                    zT_cur, hT_cur, h_lane = zT_nxt, hT_nxt, h_new

            # ---------------- phase 3: head ----------------
            with (
                tc.tile_pool(name="p3", bufs=1) as p3,
                tc.tile_pool(name="p3p", bufs=1, space="PSUM") as p3p,
            ):
                prod = p3.tile([128, 4, S, BS], dt, tag="prod")
                nc.vector.tensor_mul(
                    prod[:], zfin[:],
                    swfcT[:].unsqueeze(3).broadcast_to([128, 4, S, BS]))
                # reduce over (c, t): view [p, b, c, t] then reduce XY
                s_sb = p3.tile([128, BS], dt, tag="ssb")
                nc.vector.tensor_reduce(
                    s_sb[:].unsqueeze(2).unsqueeze(3),
                    prod[:].rearrange("p c t b -> p b c t"),
                    axis=mybir.AxisListType.XY, op=mybir.AluOpType.add)
                head_ps = p3p.tile([BS, 1], dt, tag="head")
                nc.tensor.matmul(head_ps[:], s_sb[:], ones_col[:],
                                 start=True, stop=True)
                res = p3.tile([BS, 1], dt, tag="res")
                nc.vector.tensor_add(res[:], head_ps[:], sbfc[:])
                nc.sync.dma_start(out_e[:], res[:])
    nc.finalize()
    return nc


def _hmask():
    m = np.ones((128, K + 1), np.float32)
    for j in range(K):
        m[8 * j:8 * j + 8, j] = 0.0
    return m


def prep_inputs(x, Wih_x, Whh_x, bih_x, bhh_x, Wih_z, Whh_z, bih_z, bhh_z,
                Wfc, bfc):
    f = np.float32
    shared = {
        "w_rz_x": np.concatenate([Wih_x[:1024].T, Whh_x[:1024].T], 0)
        .reshape(5, 128, 1024).transpose(1, 0, 2).astype(f).copy(),
        "w_ni_x": Wih_x[1024:].T.reshape(1, 128, 512).transpose(1, 0, 2)
        .astype(f).copy(),
        "w_nh_x": Whh_x[1024:].T.reshape(4, 128, 512).transpose(1, 0, 2)
        .astype(f).copy(),
        "b_rz_x": (bih_x + bhh_x)[None, :1024].astype(f).copy(),
        "b_ni_x": bih_x[None, 1024:].astype(f).copy(),
        "b_nh_x": bhh_x[None, 1024:].astype(f).copy(),
        "w_rz": np.concatenate([Wih_z[:1024].T, Whh_z[:1024].T], 0)
        .reshape(8, 128, 1024).transpose(1, 0, 2).astype(f).copy(),
        "w_ni": Wih_z[1024:].T.reshape(4, 128, 512).transpose(1, 0, 2)
        .astype(f).copy(),
        "w_nh": Whh_z[1024:].T.reshape(4, 128, 512).transpose(1, 0, 2)
        .astype(f).copy(),
        "b_rz": (bih_z + bhh_z)[None, :1024].astype(f).copy(),
        "b_ni": bih_z[None, 1024:].astype(f).copy(),
        "b_nh": bhh_z[None, 1024:].astype(f).copy(),
        "wfcT": Wfc[0].reshape(S, 4, 128).transpose(2, 1, 0).astype(f).copy(),
        "bfc_r": np.full((BS, 1), bfc[0], f),
        "hmask": _hmask(),
    }
    in_maps = []
    for c in range(NCORE):
        m = dict(shared)
        m["xT"] = x[BS * c:BS * c + BS].transpose(2, 1, 0).astype(f).copy()
        in_maps.append(m)
    return in_maps


def run(inputs_dict, trace=False, time_second_run=False):
    import time as _time
    nc = build_nc()
    in_maps = prep_inputs(**inputs_dict)
    res = run_bass_kernel_spmd(nc, in_maps, core_ids=list(range(NCORE)),
                               trace=trace)
    out = np.concatenate([r["out"] for r in res.results], axis=0)
    wall_ns = None
    if time_second_run:
        t0 = _time.perf_counter()
        res2 = run_bass_kernel_spmd(nc, in_maps, core_ids=list(range(NCORE)),
                                    trace=False)
        wall_ns = int((_time.perf_counter() - t0) * 1e9)
        out2 = np.concatenate([r["out"] for r in res2.results], axis=0)
        assert np.allclose(out, out2, atol=1e-5)
    return out.astype(np.float32), res, wall_ns


def kernel(**inputs):
    out, _ = run(inputs, trace=False)
    return out
